# revision 19
# baseline (speedup 1.0000x reference)
"""Trainium2 Bass kernel for a BERT-style prototype classifier head
(cosine prototype similarities -> per-prototype MDN over token positions ->
density mask -> masked mean pooling -> cosine classifier), data-parallel over
batch across 8 NeuronCores.

Self-contained: hardcodes shapes B=32, L=512, H=768, P=64, K=5, C=4.
kernel(**inputs) takes the FULL numpy inputs and returns
(logits [32,4], loss_mu scalar, diversity_loss scalar).

Host-side prep is layout-only (rearrange/stack/pad/eye/arange of weights);
all model FLOPs run on-device.
"""

import numpy as np

# ---- problem constants (hardcoded per contract) ----
B = 32
L = 512
H = 768
NP = 64  # prototypes
K = 5  # mixture components
C = 4  # classes
N_CORES = 8
B_LOC = B // N_CORES  # 4 batches per core
P_ = 128  # SBUF partitions
NL = L // P_  # 4 l-chunks
NH = H // P_  # 6 h-chunks
HHALF = H // 2  # 384 (fits one PSUM bank in fp32)
LOG2PI = float(np.log(2.0 * np.pi))

_CACHE = {}


def _build_nc():
    import os

    import concourse.mybir as mybir
    import concourse.tile as tile
    from concourse import bacc

    STAGE = int(os.environ.get("KSTAGE", "7"))

    f32 = mybir.dt.float32
    AF = mybir.ActivationFunctionType
    OP = mybir.AluOpType
    AX = mybir.AxisListType

    nc = bacc.Bacc("TRN2", target_bir_lowering=False, debug=False)

    emb_d = nc.dram_tensor("emb", [B_LOC, L, H], f32, kind="ExternalInput")
    hid_d = nc.dram_tensor("hid", [B_LOC, L, H], f32, kind="ExternalInput")
    protos2_d = nc.dram_tensor("protos2", [P_, H], f32, kind="ExternalInput")
    w1sb_d = nc.dram_tensor("w1sb", [P_, NL, L], f32, kind="ExternalInput")
    whead_d = nc.dram_tensor("whead", [P_, NL, 3 * K], f32, kind="ExternalInput")
    whpad_d = nc.dram_tensor("whpad", [P_, 3 * K], f32, kind="ExternalInput")
    b1cols_d = nc.dram_tensor("b1cols", [P_, NL], f32, kind="ExternalInput")
    ident_d = nc.dram_tensor("ident", [P_, P_], f32, kind="ExternalInput")
    onespad_d = nc.dram_tensor("onespad", [P_, P_], f32, kind="ExternalInput")
    ones_d = nc.dram_tensor("ones_t", [P_, NP], f32, kind="ExternalInput")
    tvals_d = nc.dram_tensor("tvals", [P_, L], f32, kind="ExternalInput")
    inv_eye_d = nc.dram_tensor("inv_eye", [NP, NP], f32, kind="ExternalInput")
    fcwpad2_d = nc.dram_tensor("fcwpad2", [P_, 2 * C], f32, kind="ExternalInput")
    fcb2_d = nc.dram_tensor("fcb2", [P_, 2 * C], f32, kind="ExternalInput")

    out_logits = nc.dram_tensor("out_logits", [2, 2 * C], f32, kind="ExternalOutput")
    out_ll = nc.dram_tensor("out_ll", [1, 1], f32, kind="ExternalOutput")
    out_div = nc.dram_tensor("out_div", [1, 1], f32, kind="ExternalOutput")
    out_dbg = None
    if STAGE < 7:
        out_dbg = nc.dram_tensor("out_dbg", [P_, L], f32, kind="ExternalOutput")

    with tile.TileContext(nc) as tc:
        with (
            tc.tile_pool(name="const", bufs=1) as cpool,
            tc.tile_pool(name="an", bufs=2) as an_pool,
            tc.tile_pool(name="embT", bufs=2) as embT_pool,
            tc.tile_pool(name="hid", bufs=4) as hid_pool,
            tc.tile_pool(name="grp", bufs=2) as grp_pool,
            tc.tile_pool(name="scr", bufs=2) as scr_pool,
            tc.tile_pool(name="small", bufs=2) as sm_pool,
            tc.tile_pool(name="ptr", bufs=2, space="PSUM") as ptr_pool,
            tc.tile_pool(name="pmm", bufs=3, space="PSUM") as pmm_pool,
            tc.tile_pool(name="pll", bufs=1, space="PSUM") as pll_pool,
        ):
            # ---------------- constants (single clean DMAs) ----------------
            identity = cpool.tile([P_, P_], f32)
            nc.sync.dma_start(identity[:], ident_d.ap())
            w1sb = cpool.tile([P_, NL, L], f32)
            nc.sync.dma_start(w1sb[:], w1sb_d.ap())
            whead = cpool.tile([P_, NL, 3 * K], f32)
            nc.sync.dma_start(whead[:], whead_d.ap())
            whpad = cpool.tile([P_, 3 * K], f32)
            nc.sync.dma_start(whpad[:], whpad_d.ap())
            b1cols = cpool.tile([P_, NL], f32)
            nc.sync.dma_start(b1cols[:], b1cols_d.ap())
            onespad = cpool.tile([P_, P_], f32)
            nc.sync.dma_start(onespad[:], onespad_d.ap())
            ones_t = cpool.tile([P_, NP], f32)
            nc.sync.dma_start(ones_t[:], ones_d.ap())
            tvals = cpool.tile([P_, L], f32)
            nc.sync.dma_start(tvals[:], tvals_d.ap())
            inv_eye = cpool.tile([NP, NP], f32)
            nc.sync.dma_start(inv_eye[:], inv_eye_d.ap())
            fcwpad2 = cpool.tile([P_, 2 * C], f32)
            nc.sync.dma_start(fcwpad2[:], fcwpad2_d.ap())
            fcb2 = cpool.tile([P_, 2 * C], f32)
            nc.sync.dma_start(fcb2[:], fcb2_d.ap())
            protos2 = cpool.tile([P_, H], f32)
            nc.sync.dma_start(protos2[:], protos2_d.ap())

            # const APs used by scalar.activation for float biases
            zero_col = cpool.tile([P_, 1], f32)
            nc.vector.memset(zero_col[:], 0.0)
            half_col = cpool.tile([P_, 1], f32)
            nc.vector.memset(half_col[:], 0.5)
            nc.const_aps.aps[(f32, 0.0)] = zero_col[:]
            nc.const_aps.aps[(f32, 0.5)] = half_col[:]

            # prototype norms + normalized prototypes
            psq = scr_pool.tile([P_, H], f32, tag="sq768")
            ssp = cpool.tile([P_, 1], f32)
            nc.scalar.activation(psq[:], protos2[:], AF.Square, accum_out=ssp[:])
            normp = cpool.tile([P_, 1], f32)
            nc.scalar.activation(normp[:], ssp[:], AF.Sqrt)
            nc.vector.tensor_scalar_max(normp[:], normp[:], 1e-12)
            invnp = cpool.tile([P_, 1], f32)
            nc.vector.reciprocal(invnp[:], normp[:])
            pn2 = cpool.tile([P_, H], f32)
            nc.vector.tensor_scalar_mul(pn2[:], protos2[:], invnp[:, 0:1])

            # pnT[:, jh, :] = normalized prototypes transposed  [H-chunk, NP]
            # protoT likewise unnormalized (for the diversity Gram)
            pnT = cpool.tile([P_, NH, NP], f32)
            protoT = cpool.tile([P_, NH, NP], f32)
            for src, dst in ((pn2, pnT), (protos2, protoT)):
                for jh in range(NH):
                    ptr = ptr_pool.tile([P_, NP], f32, tag="tr64")
                    nc.tensor.transpose(
                        ptr[:],
                        src[0:NP, jh * P_ : (jh + 1) * P_],
                        identity[0:NP, 0:NP],
                    )
                    nc.vector.tensor_copy(dst[:, jh, :], ptr[:])

            sqT = cpool.tile([P_, NH, NP], f32)
            nc.scalar.activation(sqT[:], protoT[:], AF.Square)

            # ---------------- diversity loss (prototypes only) ----------------
            ps_g = pmm_pool.tile([NP, NP], f32, tag="mm")
            for jh in range(NH):
                nc.tensor.matmul(
                    ps_g[:],
                    protoT[:, jh, :],
                    protoT[:, jh, :],
                    start=(jh == 0),
                    stop=(jh == NH - 1),
                )
            ps_o = pmm_pool.tile([NP, NP], f32, tag="mm")
            for jh in range(NH):
                nc.tensor.matmul(
                    ps_o[:],
                    ones_t[:],
                    sqT[:, jh, :],
                    start=(jh == 0),
                    stop=(jh == NH - 1),
                )
            d2 = sm_pool.tile([NP, NP], f32, tag="d2")
            nc.vector.tensor_scalar_add(d2[:], ps_o[:], ssp[0:NP, 0:1])
            gm2 = sm_pool.tile([NP, NP], f32, tag="d2b")
            nc.vector.tensor_scalar_mul(gm2[:], ps_g[:], -2.0)
            nc.vector.tensor_add(d2[:], d2[:], gm2[:])
            nc.vector.tensor_scalar_max(d2[:], d2[:], 1e-12)
            nc.scalar.activation(d2[:], d2[:], AF.Sqrt)
            # relu(0.5 - D), zero the diagonal
            nc.scalar.activation(d2[:], d2[:], AF.Relu, bias=0.5, scale=-1.0)
            nc.vector.tensor_mul(d2[:], d2[:], inv_eye[:])
            rdsum = sm_pool.tile([NP, 1], f32, tag="rdsum")
            nc.vector.reduce_sum(rdsum[:], d2[:], axis=AX.X)
            ps_div = pmm_pool.tile([1, 1], f32, tag="mm")
            nc.tensor.matmul(ps_div[:], rdsum[:], ones_t[0:NP, 0:1])
            divsb = sm_pool.tile([1, 1], f32, tag="div")
            nc.vector.tensor_scalar_mul(divsb[:], ps_div[:], 0.5)
            nc.sync.dma_start(out_div.ap(), divsb[:])

            # ---------------- main pipeline ----------------
            ll_ps = pll_pool.tile([1, 1], f32, tag="ll")
            simall2 = cpool.tile([P_, 2], f32)
            hid_tiles = {}
            group_state = {}

            for b in range(B_LOC):
                g, hb = divmod(b, 2)

                # load inputs for this batch
                an = an_pool.tile([P_, NL, H], f32, tag="an")
                nc.sync.dma_start(
                    an[:], emb_d.ap()[b].rearrange("(lc p) h -> p lc h", p=P_)
                )
                hsb = hid_pool.tile([P_, NL, H], f32, tag="hid")
                nc.sync.dma_start(
                    hsb[:], hid_d.ap()[b].rearrange("(lc p) h -> p lc h", p=P_)
                )
                hid_tiles[b] = hsb

                # row norms over H, then normalize in place
                ss4 = sm_pool.tile([P_, NL], f32, tag="ss4")
                for lc in range(NL):
                    sq = scr_pool.tile([P_, H], f32, tag="sq768")
                    nc.scalar.activation(
                        sq[:], an[:, lc, :], AF.Square, accum_out=ss4[:, lc : lc + 1]
                    )
                nc.scalar.activation(ss4[:], ss4[:], AF.Sqrt)
                nc.vector.tensor_scalar_max(ss4[:], ss4[:], 1e-12)
                invn4 = sm_pool.tile([P_, NL], f32, tag="invn4")
                nc.vector.reciprocal(invn4[:], ss4[:])
                for lc in range(NL):
                    nc.vector.tensor_scalar_mul(
                        an[:, lc, :], an[:, lc, :], invn4[:, lc : lc + 1]
                    )

                # transpose normalized embeddings: embT[:, jh, lc*128:...] = a_n^T
                embT = embT_pool.tile([P_, NH, L], f32, tag="embT")
                for jh in range(NH):
                    ptr = ptr_pool.tile([P_, NL, P_], f32, tag="tr")
                    for lc in range(NL):
                        nc.tensor.transpose(
                            ptr[:, lc, :],
                            an[:, lc, jh * P_ : (jh + 1) * P_],
                            identity[:],
                        )
                    nc.vector.tensor_copy(
                        embT[:, jh, :], ptr.rearrange("p a b -> p (a b)")
                    )

                # cosine similarities, prototype-major: simP rows = (batch, proto)
                if hb == 0:
                    ps_sim = pmm_pool.tile([P_, L], f32, tag="mm")
                    group_state[g] = ps_sim
                else:
                    ps_sim = group_state[g]
                for jh in range(NH):
                    nc.tensor.matmul(
                        ps_sim[hb * NP : (hb + 1) * NP, :],
                        pnT[:, jh, :],
                        embT[:, jh, :],
                        start=(jh == 0),
                        stop=(jh == NH - 1),
                    )

                if hb != 1:
                    continue

                # ---------------- per-group (128 rows) ----------------
                simP = grp_pool.tile([P_, L], f32, tag="simP")
                nc.vector.tensor_copy(simP[:], ps_sim[:])

                if STAGE < 5 and g == 0:
                    nc.sync.dma_start(out_dbg.ap(), simP[:])
                if STAGE < 2:
                    continue

                # top-K token positions per row (MDN labels)
                vmax8 = sm_pool.tile([P_, 8], f32, tag="vmax8")
                nc.vector.max(out=vmax8[:], in_=simP[:])
                idx8 = sm_pool.tile([P_, 8], mybir.dt.uint32, tag="idx8")
                nc.vector.max_index(out=idx8[:], in_max=vmax8[:], in_values=simP[:])
                y5 = sm_pool.tile([P_, K], f32, tag="y5")
                nc.vector.tensor_copy(y5[:], idx8[:, 0:K])

                if STAGE < 3:
                    continue

                # csT tiles for the W1 matmul: csT[:, lc, :] = simP^T chunk
                csT = grp_pool.tile([P_, NL, P_], f32, tag="csT")
                ptr = ptr_pool.tile([P_, NL, P_], f32, tag="tr")
                for lc in range(NL):
                    nc.tensor.transpose(
                        ptr[:, lc, :], simP[:, lc * P_ : (lc + 1) * P_], identity[:]
                    )
                nc.vector.tensor_copy(
                    csT.rearrange("p a b -> p (a b)"), ptr.rearrange("p a b -> p (a b)")
                )

                # hT = tanh(W1^T cs^T + b1) in transposed layout [h-unit, row]
                ps_h = pmm_pool.tile([P_, NL, P_], f32, tag="mm")
                for j in range(NL):
                    for ko in range(NL):
                        nc.tensor.matmul(
                            ps_h[:, j, :],
                            w1sb[:, ko, j * P_ : (j + 1) * P_],
                            csT[:, ko, :],
                            start=(ko == 0),
                            stop=(ko == NL - 1),
                        )
                hT = grp_pool.tile([P_, NL, P_], f32, tag="hT")
                for j in range(NL):
                    nc.scalar.activation(
                        hT[:, j, :],
                        ps_h[:, j, :],
                        AF.Tanh,
                        bias=b1cols[:, j : j + 1],
                    )

                # MDN heads: [row, 15] = [log-pi-logits | mu | log-sigma]
                ps_hd = pmm_pool.tile([P_, 3 * K], f32, tag="mm")
                for ko in range(NL):
                    nc.tensor.matmul(
                        ps_hd[:],
                        hT[:, ko, :],
                        whead[:, ko, :],
                        start=(ko == 0),
                        stop=False,
                    )
                nc.tensor.matmul(ps_hd[:], onespad[:], whpad[:], start=False, stop=True)
                hd = sm_pool.tile([P_, 3 * K], f32, tag="hd")
                nc.vector.tensor_copy(hd[:], ps_hd[:])

                x5 = hd[:, 0:K]
                mu5 = hd[:, K : 2 * K]
                lsg5 = hd[:, 2 * K : 3 * K]

                # log-softmax over K
                m1 = sm_pool.tile([P_, 1], f32, tag="m1")
                nc.vector.reduce_max(m1[:], x5, axis=AX.X)
                negm = sm_pool.tile([P_, 1], f32, tag="negm")
                nc.vector.tensor_scalar_mul(negm[:], m1[:], -1.0)
                e5 = sm_pool.tile([P_, K], f32, tag="e5")
                nc.scalar.activation(e5[:], x5, AF.Exp, bias=negm[:, 0:1])
                s1 = sm_pool.tile([P_, 1], f32, tag="s1")
                nc.vector.reduce_sum(s1[:], e5[:], axis=AX.X)
                nc.scalar.activation(s1[:], s1[:], AF.Ln)
                nc.vector.tensor_add(s1[:], s1[:], m1[:])  # logZ
                logpi5 = sm_pool.tile([P_, K], f32, tag="logpi")
                nc.vector.tensor_scalar(logpi5[:], x5, s1[:, 0:1], None, OP.subtract)

                invsig5 = sm_pool.tile([P_, K], f32, tag="invsig")
                nc.scalar.activation(invsig5[:], lsg5, AF.Exp, scale=-1.0)
                nmui5 = sm_pool.tile([P_, K], f32, tag="nmui")
                nc.vector.tensor_mul(nmui5[:], mu5, invsig5[:])
                nc.vector.tensor_scalar_mul(nmui5[:], nmui5[:], -1.0)
                # w5 = log_pi - log_sigma - 0.5*log(2*pi)
                w5 = sm_pool.tile([P_, K], f32, tag="w5")
                nc.vector.tensor_sub(w5[:], logpi5[:], lsg5)
                nc.vector.tensor_scalar_add(w5[:], w5[:], -0.5 * LOG2PI)

                # MDN NLL: score each of the K labels under the full mixture
                z55 = sm_pool.tile([P_, K, K], f32, tag="z55")
                y_bc = y5.rearrange("p (j o) -> p j o", o=1).to_broadcast([P_, K, K])
                mu_bc = mu5.rearrange("p (o k) -> p o k", o=1).to_broadcast([P_, K, K])
                is_bc = invsig5.rearrange("p (o k) -> p o k", o=1).to_broadcast(
                    [P_, K, K]
                )
                w_bc = w5.rearrange("p (o k) -> p o k", o=1).to_broadcast([P_, K, K])
                nc.vector.tensor_tensor(z55[:], y_bc, mu_bc, OP.subtract)
                nc.vector.tensor_tensor(z55[:], z55[:], is_bc, OP.mult)
                nc.scalar.activation(z55[:], z55[:], AF.Square)
                nc.vector.tensor_scalar_mul(z55[:], z55[:], -0.5)
                nc.vector.tensor_tensor(z55[:], z55[:], w_bc, OP.add)
                mx5 = sm_pool.tile([P_, K], f32, tag="mx5")
                nc.vector.reduce_max(mx5[:], z55[:], axis=AX.X)
                mx_bc = mx5.rearrange("p (j o) -> p j o", o=1).to_broadcast([P_, K, K])
                nc.vector.tensor_tensor(z55[:], z55[:], mx_bc, OP.subtract)
                nc.scalar.activation(z55[:], z55[:], AF.Exp)
                ll5 = sm_pool.tile([P_, K], f32, tag="ll5")
                nc.vector.reduce_sum(ll5[:], z55[:], axis=AX.X)
                nc.scalar.activation(ll5[:], ll5[:], AF.Ln)
                nc.vector.tensor_add(ll5[:], ll5[:], mx5[:])
                llrow = sm_pool.tile([P_, 1], f32, tag="llrow")
                nc.vector.reduce_sum(llrow[:], ll5[:], axis=AX.X)
                nc.tensor.matmul(
                    ll_ps[:],
                    llrow[:],
                    ones_t[:, 0:1],
                    start=(g == 0),
                    stop=(g == B_LOC // 2 - 1),
                )

                if STAGE < 4:
                    continue

                # density mask over token positions: dens[row, t]
                dens = grp_pool.tile([P_, L], f32, tag="dens")
                u2 = scr_pool.tile([P_, L], f32, tag="u2")
                gk = scr_pool.tile([P_, L], f32, tag="gk")
                for k in range(K):
                    nc.scalar.activation(
                        u2[:],
                        tvals[:],
                        AF.Square,
                        scale=invsig5[:, k : k + 1],
                        bias=nmui5[:, k : k + 1],
                    )
                    if k == 0:
                        nc.scalar.activation(
                            dens[:], u2[:], AF.Exp, scale=-0.5, bias=w5[:, 0:1]
                        )
                    else:
                        nc.scalar.activation(
                            gk[:], u2[:], AF.Exp, scale=-0.5, bias=w5[:, k : k + 1]
                        )
                        nc.vector.tensor_add(dens[:], dens[:], gk[:])

                denom = sm_pool.tile([P_, 1], f32, tag="denom")
                nc.vector.reduce_sum(denom[:], dens[:], axis=AX.X)
                nc.vector.tensor_scalar_max(denom[:], denom[:], 1e-9)
                zscale = sm_pool.tile([P_, 1], f32, tag="zscale")
                nc.vector.reciprocal(zscale[:], denom[:])
                nc.vector.tensor_scalar_mul(zscale[:], zscale[:], 1.0 / L)

                # mask transposed for pooling: maskT[:, lc, :] = dens^T chunk
                maskT = grp_pool.tile([P_, NL, P_], f32, tag="maskT")
                ptr2 = ptr_pool.tile([P_, NL, P_], f32, tag="tr")
                for lc in range(NL):
                    nc.tensor.transpose(
                        ptr2[:, lc, :], dens[:, lc * P_ : (lc + 1) * P_], identity[:]
                    )
                nc.vector.tensor_copy(
                    maskT.rearrange("p a b -> p (a b)"),
                    ptr2.rearrange("p a b -> p (a b)"),
                )

                if STAGE < 5:
                    continue

                # masked mean pooling Z[row, h] (both batches of the group)
                zsb = grp_pool.tile([P_, 2, HHALF], f32, tag="zsb")
                for half in range(2):
                    ps_z = pmm_pool.tile([P_, HHALF], f32, tag="mm")
                    for hh in range(2):
                        for ko in range(NL):
                            nc.tensor.matmul(
                                ps_z[hh * NP : (hh + 1) * NP, :],
                                maskT[:, ko, hh * NP : (hh + 1) * NP],
                                hid_tiles[2 * g + hh][
                                    :, ko, half * HHALF : (half + 1) * HHALF
                                ],
                                start=(ko == 0),
                                stop=(ko == NL - 1),
                            )
                    nc.vector.tensor_scalar_mul(zsb[:, half, :], ps_z[:], zscale[:, 0:1])

                if STAGE == 5 and g == 0:
                    nc.sync.dma_start(out_dbg.ap()[:, 0:HHALF], zsb[:, 0, :])
                if STAGE < 6:
                    continue

                # cosine similarity of pooled Z rows to prototypes
                numh = sm_pool.tile([P_, 2], f32, tag="numh")
                zssh = sm_pool.tile([P_, 2], f32, tag="zssh")
                for half in range(2):
                    scrb = scr_pool.tile([P_, HHALF], f32, tag="scrb")
                    nc.vector.tensor_mul(
                        scrb[:],
                        zsb[:, half, :],
                        protos2[:, half * HHALF : (half + 1) * HHALF],
                    )
                    nc.vector.reduce_sum(
                        numh[:, half : half + 1], scrb[:], axis=AX.X
                    )
                    scrb2 = scr_pool.tile([P_, HHALF], f32, tag="scrb")
                    nc.scalar.activation(
                        scrb2[:],
                        zsb[:, half, :],
                        AF.Square,
                        accum_out=zssh[:, half : half + 1],
                    )
                num = sm_pool.tile([P_, 1], f32, tag="num")
                nc.vector.reduce_sum(num[:], numh[:], axis=AX.X)
                zss = sm_pool.tile([P_, 1], f32, tag="zss")
                nc.vector.reduce_sum(zss[:], zssh[:], axis=AX.X)
                nc.scalar.activation(zss[:], zss[:], AF.Sqrt)
                nc.vector.tensor_mul(zss[:], zss[:], normp[:])
                nc.vector.tensor_scalar_max(zss[:], zss[:], 1e-6)
                nc.vector.reciprocal(zss[:], zss[:])
                nc.vector.tensor_mul(num[:], num[:], zss[:])
                nc.vector.tensor_copy(simall2[:, g : g + 1], num[:])

            if STAGE >= 7:
                # classifier head: one matmul for all 4 batches
                ps_lg = pmm_pool.tile([2, 2 * C], f32, tag="mm")
                nc.tensor.matmul(
                    ps_lg[:], simall2[:], fcwpad2[:], start=True, stop=False
                )
                nc.tensor.matmul(
                    ps_lg[:], onespad[:, 0:2], fcb2[:], start=False, stop=True
                )
                lgsb = sm_pool.tile([2, 2 * C], f32, tag="lg")
                nc.vector.tensor_copy(lgsb[:], ps_lg[:])
                nc.sync.dma_start(out_logits.ap(), lgsb[:])

            if STAGE >= 3:
                llsb = sm_pool.tile([1, 1], f32, tag="llsb")
                nc.vector.tensor_copy(llsb[:], ll_ps[:])
                nc.sync.dma_start(out_ll.ap(), llsb[:])

    nc.compile()
    return nc


def get_nc():
    if "nc" not in _CACHE:
        _CACHE["nc"] = _build_nc()
    return _CACHE["nc"]


def _host_consts(inputs):
    """Layout-only host prep of the replicated parameters."""
    f = np.float32
    protos = np.asarray(inputs["prototype_vectors"], f)
    W1 = np.asarray(inputs["W1"], f)
    b1 = np.asarray(inputs["b1"], f)
    Wpi = np.asarray(inputs["Wpi"], f)
    Wmu = np.asarray(inputs["Wmu"], f)
    Wsig = np.asarray(inputs["Wsig"], f)
    bpi = np.asarray(inputs["bpi"], f)
    bmu = np.asarray(inputs["bmu"], f)
    bsig = np.asarray(inputs["bsig"], f)
    fcW = np.asarray(inputs["fcW"], f)
    fcb = np.asarray(inputs["fcb"], f)

    whpad = np.zeros((P_, 3 * K), f)
    whpad[0, :] = np.concatenate([bpi, bmu, bsig])
    onespad = np.zeros((P_, P_), f)
    onespad[0, :] = 1.0
    fcwpad2 = np.zeros((P_, 2 * C), f)
    fcwpad2[0:NP, 0:C] = fcW
    fcwpad2[NP:P_, C : 2 * C] = fcW
    fcb2 = np.zeros((P_, 2 * C), f)
    fcb2[0, 0:C] = fcb
    fcb2[0, C : 2 * C] = fcb

    return {
        "protos2": np.ascontiguousarray(np.concatenate([protos, protos], axis=0)),
        "w1sb": np.ascontiguousarray(
            W1.reshape(NL, P_, L).transpose(1, 0, 2)
        ),
        "whead": np.ascontiguousarray(
            np.concatenate([Wpi, Wmu, Wsig], axis=1)
            .reshape(NL, P_, 3 * K)
            .transpose(1, 0, 2)
        ),
        "whpad": whpad,
        "b1cols": np.ascontiguousarray(b1.reshape(NL, P_).T),
        "ident": np.eye(P_, dtype=f),
        "onespad": onespad,
        "ones_t": np.ones((P_, NP), f),
        "tvals": np.ascontiguousarray(
            np.broadcast_to(np.arange(L, dtype=f), (P_, L))
        ),
        "inv_eye": np.ascontiguousarray(1.0 - np.eye(NP, dtype=f)),
        "fcwpad2": fcwpad2,
        "fcb2": fcb2,
    }


def make_in_maps(inputs):
    """Shard the full inputs batch-wise across the 8 cores."""
    emb = np.ascontiguousarray(inputs["padded_emb"], dtype=np.float32)
    hid = np.ascontiguousarray(inputs["hidden_states"], dtype=np.float32)
    shared = _host_consts(inputs)
    in_maps = []
    for c in range(N_CORES):
        m = dict(shared)
        m["emb"] = np.ascontiguousarray(emb[c * B_LOC : (c + 1) * B_LOC])
        m["hid"] = np.ascontiguousarray(hid[c * B_LOC : (c + 1) * B_LOC])
        in_maps.append(m)
    return in_maps


def assemble(results):
    """Combine per-core outputs into the full (logits, loss_mu, diversity)."""
    logits = np.concatenate(
        [r["out_logits"].reshape(2, 2, C).reshape(B_LOC, C) for r in results], axis=0
    )
    ll_total = float(sum(r["out_ll"][0, 0] for r in results))
    loss_mu = np.float32(-ll_total / (B * NP * K))
    diversity = np.float32(results[0]["out_div"][0, 0])
    return logits.astype(np.float32), loss_mu, diversity


def run_on_hw(inputs, trace=False):
    from concourse.bass_utils import run_bass_kernel_spmd

    nc = get_nc()
    res = run_bass_kernel_spmd(
        nc, make_in_maps(inputs), core_ids=list(range(N_CORES)), trace=trace
    )
    return assemble(res.results), res


def kernel(**inputs):
    (out, _res) = run_on_hw(inputs, trace=False)
    return out


# revision 24
# speedup vs baseline: 1.0221x; 1.0221x over previous
"""Trainium2 Bass kernel for a BERT-style prototype classifier head
(cosine prototype similarities -> per-prototype MDN over token positions ->
density mask -> masked mean pooling -> cosine classifier), data-parallel over
batch across 8 NeuronCores.

Self-contained: hardcodes shapes B=32, L=512, H=768, P=64, K=5, C=4.
kernel(**inputs) takes the FULL numpy inputs and returns
(logits [32,4], loss_mu scalar, diversity_loss scalar).

Host-side prep is layout-only (rearrange/stack/pad/eye/arange of weights);
all model FLOPs run on-device.
"""

import numpy as np

# ---- problem constants (hardcoded per contract) ----
B = 32
L = 512
H = 768
NP = 64  # prototypes
K = 5  # mixture components
C = 4  # classes
N_CORES = 8
B_LOC = B // N_CORES  # 4 batches per core
P_ = 128  # SBUF partitions
NL = L // P_  # 4 l-chunks
NH = H // P_  # 6 h-chunks
HHALF = H // 2  # 384 (fits one PSUM bank in fp32)
LOG2PI = float(np.log(2.0 * np.pi))

_CACHE = {}


def _build_nc():
    import os

    import concourse.mybir as mybir
    import concourse.tile as tile
    from concourse import bacc

    STAGE = int(os.environ.get("KSTAGE", "7"))

    f32 = mybir.dt.float32
    AF = mybir.ActivationFunctionType
    OP = mybir.AluOpType
    AX = mybir.AxisListType

    nc = bacc.Bacc("TRN2", target_bir_lowering=False, debug=False)

    emb_d = nc.dram_tensor("emb", [B_LOC, L, H], f32, kind="ExternalInput")
    hid_d = nc.dram_tensor("hid", [B_LOC, L, H], f32, kind="ExternalInput")
    protos2_d = nc.dram_tensor("protos2", [P_, H], f32, kind="ExternalInput")
    w1sb_d = nc.dram_tensor("w1sb", [P_, NL, L], f32, kind="ExternalInput")
    whead_d = nc.dram_tensor("whead", [P_, NL, 3 * K], f32, kind="ExternalInput")
    whpad_d = nc.dram_tensor("whpad", [P_, 3 * K], f32, kind="ExternalInput")
    b1cols_d = nc.dram_tensor("b1cols", [P_, NL], f32, kind="ExternalInput")
    ident_d = nc.dram_tensor("ident", [P_, P_], f32, kind="ExternalInput")
    onespad_d = nc.dram_tensor("onespad", [P_, P_], f32, kind="ExternalInput")
    ones_d = nc.dram_tensor("ones_t", [P_, NP], f32, kind="ExternalInput")
    tvals_d = nc.dram_tensor("tvals", [P_, L], f32, kind="ExternalInput")
    inv_eye_d = nc.dram_tensor("inv_eye", [NP, NP], f32, kind="ExternalInput")
    fcwpad2_d = nc.dram_tensor("fcwpad2", [P_, 2 * C], f32, kind="ExternalInput")
    fcb2_d = nc.dram_tensor("fcb2", [P_, 2 * C], f32, kind="ExternalInput")

    out_logits = nc.dram_tensor("out_logits", [2, 2 * C], f32, kind="ExternalOutput")
    out_ll = nc.dram_tensor("out_ll", [1, 1], f32, kind="ExternalOutput")
    out_div = nc.dram_tensor("out_div", [1, 1], f32, kind="ExternalOutput")
    out_dbg = None
    if STAGE < 7:
        out_dbg = nc.dram_tensor("out_dbg", [P_, L], f32, kind="ExternalOutput")

    with tile.TileContext(nc) as tc:
        with (
            tc.tile_pool(name="const", bufs=1) as cpool,
            tc.tile_pool(name="an", bufs=2) as an_pool,
            tc.tile_pool(name="embT", bufs=2) as embT_pool,
            tc.tile_pool(name="hid", bufs=4) as hid_pool,
            tc.tile_pool(name="grp", bufs=2) as grp_pool,
            tc.tile_pool(name="scr", bufs=2) as scr_pool,
            tc.tile_pool(name="small", bufs=2) as sm_pool,
            tc.tile_pool(name="ptr", bufs=2, space="PSUM") as ptr_pool,
            tc.tile_pool(name="pmm", bufs=4, space="PSUM") as pmm_pool,
            tc.tile_pool(name="pll", bufs=1, space="PSUM") as pll_pool,
        ):
            # ---------------- constants (single clean DMAs) ----------------
            identity = cpool.tile([P_, P_], f32)
            nc.sync.dma_start(identity[:], ident_d.ap())
            w1sb = cpool.tile([P_, NL, L], f32)
            nc.sync.dma_start(w1sb[:], w1sb_d.ap())
            whead = cpool.tile([P_, NL, 3 * K], f32)
            nc.sync.dma_start(whead[:], whead_d.ap())
            whpad = cpool.tile([P_, 3 * K], f32)
            nc.sync.dma_start(whpad[:], whpad_d.ap())
            b1cols = cpool.tile([P_, NL], f32)
            nc.sync.dma_start(b1cols[:], b1cols_d.ap())
            onespad = cpool.tile([P_, P_], f32)
            nc.sync.dma_start(onespad[:], onespad_d.ap())
            ones_t = cpool.tile([P_, NP], f32)
            nc.sync.dma_start(ones_t[:], ones_d.ap())
            tvals = cpool.tile([P_, L], f32)
            nc.sync.dma_start(tvals[:], tvals_d.ap())
            inv_eye = cpool.tile([NP, NP], f32)
            nc.sync.dma_start(inv_eye[:], inv_eye_d.ap())
            fcwpad2 = cpool.tile([P_, 2 * C], f32)
            nc.sync.dma_start(fcwpad2[:], fcwpad2_d.ap())
            fcb2 = cpool.tile([P_, 2 * C], f32)
            nc.sync.dma_start(fcb2[:], fcb2_d.ap())
            protos2 = cpool.tile([P_, H], f32)
            nc.sync.dma_start(protos2[:], protos2_d.ap())

            # const APs used by scalar.activation for float biases
            zero_col = cpool.tile([P_, 1], f32)
            nc.vector.memset(zero_col[:], 0.0)
            half_col = cpool.tile([P_, 1], f32)
            nc.vector.memset(half_col[:], 0.5)
            nc.const_aps.aps[(f32, 0.0)] = zero_col[:]
            nc.const_aps.aps[(f32, 0.5)] = half_col[:]

            # prototype norms + normalized prototypes
            psq = scr_pool.tile([P_, H], f32, tag="sq768")
            ssp = cpool.tile([P_, 1], f32)
            nc.scalar.activation(psq[:], protos2[:], AF.Square, accum_out=ssp[:])
            normp = cpool.tile([P_, 1], f32)
            nc.scalar.activation(normp[:], ssp[:], AF.Sqrt)
            nc.vector.tensor_scalar_max(normp[:], normp[:], 1e-12)
            invnp = cpool.tile([P_, 1], f32)
            nc.vector.reciprocal(invnp[:], normp[:])
            pn2 = cpool.tile([P_, H], f32)
            nc.vector.tensor_scalar_mul(pn2[:], protos2[:], invnp[:, 0:1])

            # pnT[:, jh, :] = normalized prototypes transposed  [H-chunk, NP]
            # protoT likewise unnormalized (for the diversity Gram)
            pnT = cpool.tile([P_, NH, NP], f32)
            protoT = cpool.tile([P_, NH, NP], f32)
            for src, dst in ((pn2, pnT), (protos2, protoT)):
                for jh in range(NH):
                    ptr = ptr_pool.tile([P_, NP], f32, tag="tr64", bufs=1)
                    nc.tensor.transpose(
                        ptr[:],
                        src[0:NP, jh * P_ : (jh + 1) * P_],
                        identity[0:NP, 0:NP],
                    )
                    nc.vector.tensor_copy(dst[:, jh, :], ptr[:])

            sqT = cpool.tile([P_, NH, NP], f32)
            nc.scalar.activation(sqT[:], protoT[:], AF.Square)

            # ---------------- diversity loss (prototypes only) ----------------
            ps_g = pmm_pool.tile([NP, NP], f32, tag="mm")
            for jh in range(NH):
                nc.tensor.matmul(
                    ps_g[:],
                    protoT[:, jh, :],
                    protoT[:, jh, :],
                    start=(jh == 0),
                    stop=(jh == NH - 1),
                )
            ps_o = pmm_pool.tile([NP, NP], f32, tag="mm")
            for jh in range(NH):
                nc.tensor.matmul(
                    ps_o[:],
                    ones_t[:],
                    sqT[:, jh, :],
                    start=(jh == 0),
                    stop=(jh == NH - 1),
                )
            d2 = sm_pool.tile([NP, NP], f32, tag="d2")
            nc.vector.tensor_scalar_add(d2[:], ps_o[:], ssp[0:NP, 0:1])
            gm2 = sm_pool.tile([NP, NP], f32, tag="d2b")
            nc.vector.tensor_scalar_mul(gm2[:], ps_g[:], -2.0)
            nc.vector.tensor_add(d2[:], d2[:], gm2[:])
            nc.vector.tensor_scalar_max(d2[:], d2[:], 1e-12)
            nc.scalar.activation(d2[:], d2[:], AF.Sqrt)
            # relu(0.5 - D), zero the diagonal
            nc.scalar.activation(d2[:], d2[:], AF.Relu, bias=0.5, scale=-1.0)
            nc.vector.tensor_mul(d2[:], d2[:], inv_eye[:])
            rdsum = sm_pool.tile([NP, 1], f32, tag="rdsum")
            nc.vector.reduce_sum(rdsum[:], d2[:], axis=AX.X)
            ps_div = pmm_pool.tile([1, 1], f32, tag="mm")
            nc.tensor.matmul(ps_div[:], rdsum[:], ones_t[0:NP, 0:1])
            divsb = sm_pool.tile([1, 1], f32, tag="div")
            nc.vector.tensor_scalar_mul(divsb[:], ps_div[:], 0.5)
            nc.sync.dma_start(out_div.ap(), divsb[:])

            # ---------------- main pipeline ----------------
            ll_ps = pll_pool.tile([1, 1], f32, tag="ll")
            simall2 = cpool.tile([P_, 2], f32)
            hid_tiles = {}
            group_state = {}

            for b in range(B_LOC):
                g, hb = divmod(b, 2)

                # load inputs for this batch
                an = an_pool.tile([P_, NL, H], f32, tag="an")
                nc.sync.dma_start(
                    an[:], emb_d.ap()[b].rearrange("(lc p) h -> p lc h", p=P_)
                )
                hsb = hid_pool.tile([P_, NL, H], f32, tag="hid")
                nc.sync.dma_start(
                    hsb[:], hid_d.ap()[b].rearrange("(lc p) h -> p lc h", p=P_)
                )
                hid_tiles[b] = hsb

                # row norms over H, then normalize in place
                ss4 = sm_pool.tile([P_, NL], f32, tag="ss4")
                for lc in range(NL):
                    sq = scr_pool.tile([P_, H], f32, tag="sq768")
                    nc.scalar.activation(
                        sq[:], an[:, lc, :], AF.Square, accum_out=ss4[:, lc : lc + 1]
                    )
                nc.scalar.activation(ss4[:], ss4[:], AF.Sqrt)
                nc.vector.tensor_scalar_max(ss4[:], ss4[:], 1e-12)
                invn4 = sm_pool.tile([P_, NL], f32, tag="invn4")
                nc.vector.reciprocal(invn4[:], ss4[:])
                for lc in range(NL):
                    nc.vector.tensor_scalar_mul(
                        an[:, lc, :], an[:, lc, :], invn4[:, lc : lc + 1]
                    )

                # transpose normalized embeddings: embT[:, jh, lc*128:...] = a_n^T
                embT = embT_pool.tile([P_, NH, L], f32, tag="embT")
                for jh in range(NH):
                    ptr = ptr_pool.tile([P_, NL, P_], f32, tag="tr")
                    for lc in range(NL):
                        nc.tensor.transpose(
                            ptr[:, lc, :],
                            an[:, lc, jh * P_ : (jh + 1) * P_],
                            identity[:],
                        )
                    nc.vector.tensor_copy(
                        embT[:, jh, :], ptr.rearrange("p a b -> p (a b)")
                    )

                # cosine similarities, prototype-major: simP rows = (batch, proto)
                if hb == 0:
                    ps_sim = pmm_pool.tile([P_, L], f32, tag="mm")
                    group_state[g] = ps_sim
                else:
                    ps_sim = group_state[g]
                for jh in range(NH):
                    nc.tensor.matmul(
                        ps_sim[hb * NP : (hb + 1) * NP, :],
                        pnT[:, jh, :],
                        embT[:, jh, :],
                        start=(jh == 0),
                        stop=(jh == NH - 1),
                    )

                if hb != 1:
                    continue

                # ---------------- per-group (128 rows) ----------------
                simP = grp_pool.tile([P_, L], f32, tag="simP")
                nc.vector.tensor_copy(simP[:], ps_sim[:])

                if STAGE < 5 and g == 0:
                    nc.sync.dma_start(out_dbg.ap(), simP[:])
                if STAGE < 2:
                    continue

                # top-K token positions per row (MDN labels)
                vmax8 = sm_pool.tile([P_, 8], f32, tag="vmax8")
                nc.vector.max(out=vmax8[:], in_=simP[:])
                idx8 = sm_pool.tile([P_, 8], mybir.dt.uint32, tag="idx8")
                nc.vector.max_index(out=idx8[:], in_max=vmax8[:], in_values=simP[:])
                y5 = sm_pool.tile([P_, K], f32, tag="y5")
                nc.vector.tensor_copy(y5[:], idx8[:, 0:K])

                if STAGE < 3:
                    continue

                # csT tiles for the W1 matmul: csT[:, lc, :] = simP^T chunk
                csT = grp_pool.tile([P_, NL, P_], f32, tag="csT")
                ptr = ptr_pool.tile([P_, NL, P_], f32, tag="tr")
                for lc in range(NL):
                    nc.tensor.transpose(
                        ptr[:, lc, :], simP[:, lc * P_ : (lc + 1) * P_], identity[:]
                    )
                nc.vector.tensor_copy(
                    csT.rearrange("p a b -> p (a b)"), ptr.rearrange("p a b -> p (a b)")
                )

                # hT = tanh(W1^T cs^T + b1) in transposed layout [h-unit, row]
                ps_h = pmm_pool.tile([P_, NL, P_], f32, tag="mm")
                for j in range(NL):
                    for ko in range(NL):
                        nc.tensor.matmul(
                            ps_h[:, j, :],
                            w1sb[:, ko, j * P_ : (j + 1) * P_],
                            csT[:, ko, :],
                            start=(ko == 0),
                            stop=(ko == NL - 1),
                        )
                hT = grp_pool.tile([P_, NL, P_], f32, tag="hT")
                for j in range(NL):
                    nc.scalar.activation(
                        hT[:, j, :],
                        ps_h[:, j, :],
                        AF.Tanh,
                        bias=b1cols[:, j : j + 1],
                    )

                # MDN heads: [row, 15] = [log-pi-logits | mu | log-sigma]
                ps_hd = pmm_pool.tile([P_, 3 * K], f32, tag="mm")
                for ko in range(NL):
                    nc.tensor.matmul(
                        ps_hd[:],
                        hT[:, ko, :],
                        whead[:, ko, :],
                        start=(ko == 0),
                        stop=False,
                    )
                nc.tensor.matmul(ps_hd[:], onespad[:], whpad[:], start=False, stop=True)
                hd = sm_pool.tile([P_, 3 * K], f32, tag="hd")
                nc.vector.tensor_copy(hd[:], ps_hd[:])

                x5 = hd[:, 0:K]
                mu5 = hd[:, K : 2 * K]
                lsg5 = hd[:, 2 * K : 3 * K]

                # log-softmax over K
                m1 = sm_pool.tile([P_, 1], f32, tag="m1")
                nc.vector.reduce_max(m1[:], x5, axis=AX.X)
                negm = sm_pool.tile([P_, 1], f32, tag="negm")
                nc.vector.tensor_scalar_mul(negm[:], m1[:], -1.0)
                e5 = sm_pool.tile([P_, K], f32, tag="e5")
                nc.scalar.activation(e5[:], x5, AF.Exp, bias=negm[:, 0:1])
                s1 = sm_pool.tile([P_, 1], f32, tag="s1")
                nc.vector.reduce_sum(s1[:], e5[:], axis=AX.X)
                nc.scalar.activation(s1[:], s1[:], AF.Ln)
                nc.vector.tensor_add(s1[:], s1[:], m1[:])  # logZ
                logpi5 = sm_pool.tile([P_, K], f32, tag="logpi")
                nc.vector.tensor_scalar(logpi5[:], x5, s1[:, 0:1], None, OP.subtract)

                invsig5 = sm_pool.tile([P_, K], f32, tag="invsig")
                nc.scalar.activation(invsig5[:], lsg5, AF.Exp, scale=-1.0)
                nmui5 = sm_pool.tile([P_, K], f32, tag="nmui")
                nc.vector.tensor_mul(nmui5[:], mu5, invsig5[:])
                nc.vector.tensor_scalar_mul(nmui5[:], nmui5[:], -1.0)
                # w5 = log_pi - log_sigma - 0.5*log(2*pi)
                w5 = sm_pool.tile([P_, K], f32, tag="w5")
                nc.vector.tensor_sub(w5[:], logpi5[:], lsg5)
                nc.vector.tensor_scalar_add(w5[:], w5[:], -0.5 * LOG2PI)

                # MDN NLL: score each of the K labels under the full mixture
                z55 = sm_pool.tile([P_, K, K], f32, tag="z55")
                y_bc = y5.rearrange("p (j o) -> p j o", o=1).to_broadcast([P_, K, K])
                mu_bc = mu5.rearrange("p (o k) -> p o k", o=1).to_broadcast([P_, K, K])
                is_bc = invsig5.rearrange("p (o k) -> p o k", o=1).to_broadcast(
                    [P_, K, K]
                )
                w_bc = w5.rearrange("p (o k) -> p o k", o=1).to_broadcast([P_, K, K])
                nc.vector.tensor_tensor(z55[:], y_bc, mu_bc, OP.subtract)
                nc.vector.tensor_tensor(z55[:], z55[:], is_bc, OP.mult)
                nc.scalar.activation(z55[:], z55[:], AF.Square)
                nc.vector.tensor_scalar_mul(z55[:], z55[:], -0.5)
                nc.vector.tensor_tensor(z55[:], z55[:], w_bc, OP.add)
                mx5 = sm_pool.tile([P_, K], f32, tag="mx5")
                nc.vector.reduce_max(mx5[:], z55[:], axis=AX.X)
                mx_bc = mx5.rearrange("p (j o) -> p j o", o=1).to_broadcast([P_, K, K])
                nc.vector.tensor_tensor(z55[:], z55[:], mx_bc, OP.subtract)
                nc.scalar.activation(z55[:], z55[:], AF.Exp)
                ll5 = sm_pool.tile([P_, K], f32, tag="ll5")
                nc.vector.reduce_sum(ll5[:], z55[:], axis=AX.X)
                nc.scalar.activation(ll5[:], ll5[:], AF.Ln)
                nc.vector.tensor_add(ll5[:], ll5[:], mx5[:])
                llrow = sm_pool.tile([P_, 1], f32, tag="llrow")
                nc.vector.reduce_sum(llrow[:], ll5[:], axis=AX.X)
                nc.tensor.matmul(
                    ll_ps[:],
                    llrow[:],
                    ones_t[:, 0:1],
                    start=(g == 0),
                    stop=(g == B_LOC // 2 - 1),
                )

                if STAGE < 4:
                    continue

                # density mask over token positions: dens[row, t].
                # All K Squares back-to-back, then all K Exps, to avoid
                # reloading the ACT function table per component.
                dens = grp_pool.tile([P_, L], f32, tag="dens")
                u2all = scr_pool.tile([P_, K, L], f32, tag="u2all")
                for k in range(K):
                    nc.scalar.activation(
                        u2all[:, k, :],
                        tvals[:],
                        AF.Square,
                        scale=invsig5[:, k : k + 1],
                        bias=nmui5[:, k : k + 1],
                    )
                for k in range(K):
                    nc.scalar.activation(
                        u2all[:, k, :],
                        u2all[:, k, :],
                        AF.Exp,
                        scale=-0.5,
                        bias=w5[:, k : k + 1],
                    )
                nc.vector.tensor_add(dens[:], u2all[:, 0, :], u2all[:, 1, :])
                for k in range(2, K):
                    nc.vector.tensor_add(dens[:], dens[:], u2all[:, k, :])

                denom = sm_pool.tile([P_, 1], f32, tag="denom")
                nc.vector.reduce_sum(denom[:], dens[:], axis=AX.X)
                nc.vector.tensor_scalar_max(denom[:], denom[:], 1e-9)
                zscale = sm_pool.tile([P_, 1], f32, tag="zscale")
                nc.vector.reciprocal(zscale[:], denom[:])
                nc.vector.tensor_scalar_mul(zscale[:], zscale[:], 1.0 / L)

                # mask transposed for pooling: maskT[:, lc, :] = dens^T chunk
                maskT = grp_pool.tile([P_, NL, P_], f32, tag="maskT")
                ptr2 = ptr_pool.tile([P_, NL, P_], f32, tag="tr")
                for lc in range(NL):
                    nc.tensor.transpose(
                        ptr2[:, lc, :], dens[:, lc * P_ : (lc + 1) * P_], identity[:]
                    )
                nc.vector.tensor_copy(
                    maskT.rearrange("p a b -> p (a b)"),
                    ptr2.rearrange("p a b -> p (a b)"),
                )

                if STAGE < 5:
                    continue

                # masked mean pooling Z[row, h] (both batches of the group)
                zsb = grp_pool.tile([P_, 2, HHALF], f32, tag="zsb")
                for half in range(2):
                    ps_z = pmm_pool.tile([P_, HHALF], f32, tag="mm")
                    for hh in range(2):
                        for ko in range(NL):
                            nc.tensor.matmul(
                                ps_z[hh * NP : (hh + 1) * NP, :],
                                maskT[:, ko, hh * NP : (hh + 1) * NP],
                                hid_tiles[2 * g + hh][
                                    :, ko, half * HHALF : (half + 1) * HHALF
                                ],
                                start=(ko == 0),
                                stop=(ko == NL - 1),
                            )
                    nc.vector.tensor_scalar_mul(zsb[:, half, :], ps_z[:], zscale[:, 0:1])

                if STAGE == 5 and g == 0:
                    nc.sync.dma_start(out_dbg.ap()[:, 0:HHALF], zsb[:, 0, :])
                if STAGE < 6:
                    continue

                # cosine similarity of pooled Z rows to prototypes
                numh = sm_pool.tile([P_, 2], f32, tag="numh")
                zssh = sm_pool.tile([P_, 2], f32, tag="zssh")
                for half in range(2):
                    scrb = scr_pool.tile([P_, HHALF], f32, tag="scrb")
                    nc.vector.tensor_mul(
                        scrb[:],
                        zsb[:, half, :],
                        protos2[:, half * HHALF : (half + 1) * HHALF],
                    )
                    nc.vector.reduce_sum(
                        numh[:, half : half + 1], scrb[:], axis=AX.X
                    )
                    scrb2 = scr_pool.tile([P_, HHALF], f32, tag="scrb")
                    nc.scalar.activation(
                        scrb2[:],
                        zsb[:, half, :],
                        AF.Square,
                        accum_out=zssh[:, half : half + 1],
                    )
                num = sm_pool.tile([P_, 1], f32, tag="num")
                nc.vector.reduce_sum(num[:], numh[:], axis=AX.X)
                zss = sm_pool.tile([P_, 1], f32, tag="zss")
                nc.vector.reduce_sum(zss[:], zssh[:], axis=AX.X)
                nc.scalar.activation(zss[:], zss[:], AF.Sqrt)
                nc.vector.tensor_mul(zss[:], zss[:], normp[:])
                nc.vector.tensor_scalar_max(zss[:], zss[:], 1e-6)
                nc.vector.reciprocal(zss[:], zss[:])
                nc.vector.tensor_mul(num[:], num[:], zss[:])
                nc.vector.tensor_copy(simall2[:, g : g + 1], num[:])

            if STAGE >= 7:
                # classifier head: one matmul for all 4 batches
                ps_lg = pmm_pool.tile([2, 2 * C], f32, tag="mm")
                nc.tensor.matmul(
                    ps_lg[:], simall2[:], fcwpad2[:], start=True, stop=False
                )
                nc.tensor.matmul(
                    ps_lg[:], onespad[:, 0:2], fcb2[:], start=False, stop=True
                )
                lgsb = sm_pool.tile([2, 2 * C], f32, tag="lg")
                nc.vector.tensor_copy(lgsb[:], ps_lg[:])
                nc.sync.dma_start(out_logits.ap(), lgsb[:])

            if STAGE >= 3:
                llsb = sm_pool.tile([1, 1], f32, tag="llsb")
                nc.vector.tensor_copy(llsb[:], ll_ps[:])
                nc.sync.dma_start(out_ll.ap(), llsb[:])

    nc.compile()
    return nc


def get_nc():
    if "nc" not in _CACHE:
        _CACHE["nc"] = _build_nc()
    return _CACHE["nc"]


def _host_consts(inputs):
    """Layout-only host prep of the replicated parameters."""
    f = np.float32
    protos = np.asarray(inputs["prototype_vectors"], f)
    W1 = np.asarray(inputs["W1"], f)
    b1 = np.asarray(inputs["b1"], f)
    Wpi = np.asarray(inputs["Wpi"], f)
    Wmu = np.asarray(inputs["Wmu"], f)
    Wsig = np.asarray(inputs["Wsig"], f)
    bpi = np.asarray(inputs["bpi"], f)
    bmu = np.asarray(inputs["bmu"], f)
    bsig = np.asarray(inputs["bsig"], f)
    fcW = np.asarray(inputs["fcW"], f)
    fcb = np.asarray(inputs["fcb"], f)

    whpad = np.zeros((P_, 3 * K), f)
    whpad[0, :] = np.concatenate([bpi, bmu, bsig])
    onespad = np.zeros((P_, P_), f)
    onespad[0, :] = 1.0
    fcwpad2 = np.zeros((P_, 2 * C), f)
    fcwpad2[0:NP, 0:C] = fcW
    fcwpad2[NP:P_, C : 2 * C] = fcW
    fcb2 = np.zeros((P_, 2 * C), f)
    fcb2[0, 0:C] = fcb
    fcb2[0, C : 2 * C] = fcb

    return {
        "protos2": np.ascontiguousarray(np.concatenate([protos, protos], axis=0)),
        "w1sb": np.ascontiguousarray(
            W1.reshape(NL, P_, L).transpose(1, 0, 2)
        ),
        "whead": np.ascontiguousarray(
            np.concatenate([Wpi, Wmu, Wsig], axis=1)
            .reshape(NL, P_, 3 * K)
            .transpose(1, 0, 2)
        ),
        "whpad": whpad,
        "b1cols": np.ascontiguousarray(b1.reshape(NL, P_).T),
        "ident": np.eye(P_, dtype=f),
        "onespad": onespad,
        "ones_t": np.ones((P_, NP), f),
        "tvals": np.ascontiguousarray(
            np.broadcast_to(np.arange(L, dtype=f), (P_, L))
        ),
        "inv_eye": np.ascontiguousarray(1.0 - np.eye(NP, dtype=f)),
        "fcwpad2": fcwpad2,
        "fcb2": fcb2,
    }


def make_in_maps(inputs):
    """Shard the full inputs batch-wise across the 8 cores."""
    emb = np.ascontiguousarray(inputs["padded_emb"], dtype=np.float32)
    hid = np.ascontiguousarray(inputs["hidden_states"], dtype=np.float32)
    shared = _host_consts(inputs)
    in_maps = []
    for c in range(N_CORES):
        m = dict(shared)
        m["emb"] = np.ascontiguousarray(emb[c * B_LOC : (c + 1) * B_LOC])
        m["hid"] = np.ascontiguousarray(hid[c * B_LOC : (c + 1) * B_LOC])
        in_maps.append(m)
    return in_maps


def assemble(results):
    """Combine per-core outputs into the full (logits, loss_mu, diversity)."""
    logits = np.concatenate(
        [r["out_logits"].reshape(2, 2, C).reshape(B_LOC, C) for r in results], axis=0
    )
    ll_total = float(sum(r["out_ll"][0, 0] for r in results))
    loss_mu = np.float32(-ll_total / (B * NP * K))
    diversity = np.float32(results[0]["out_div"][0, 0])
    return logits.astype(np.float32), loss_mu, diversity


def run_on_hw(inputs, trace=False):
    from concourse.bass_utils import run_bass_kernel_spmd

    nc = get_nc()
    res = run_bass_kernel_spmd(
        nc, make_in_maps(inputs), core_ids=list(range(N_CORES)), trace=trace
    )
    return assemble(res.results), res


def kernel(**inputs):
    (out, _res) = run_on_hw(inputs, trace=False)
    return out
